# revision 9
# baseline (speedup 1.0000x reference)
"""NuGraphCore GNN message passing on 8 trn2 NeuronCores (Bass/Tile).

Strategy (target-sharded ELL):
 - Nodes of each type (hit/sp/evt) are degree-sorted and round-robin
   assigned to 8 cores; each core owns the edges whose TARGET it owns.
 - Per block, each core processes its target tiles (128 nodes) in ELL
   layout: K_t slot columns per tile, each slot gathered from the full
   (replicated / allgathered) source table via int32 indirect DMA.
 - Softmax aggregation is done without segment-max (ratio-invariant):
   e = exp(gate*xj), aggr = sum(m*e)/sum(e), pads use -1e30 rows -> e=0.
 - Block MLPs run feature-major on PE; outputs are AllGathered between
   blocks as gather sources for the next block.
"""
import math
import numpy as np

NCORES = 8
P = 128
NEG_BIG = -1.0e30
CLAMP_LO = -55.0
KCAP = 32
DENOM_EPS = 1e-30

_CACHE = {}


# ---------------------------------------------------------------- host prep

def _rank_perm(degrees):
    """Sort nodes by (deg desc) stably; return rank per node id."""
    order = np.lexsort((np.arange(len(degrees)), -degrees))
    rank = np.empty(len(degrees), np.int64)
    rank[order] = np.arange(len(degrees))
    return rank


def _rank_perm2(deg1, deg2):
    order = np.lexsort((np.arange(len(deg1)), -deg2, -deg1))
    rank = np.empty(len(deg1), np.int64)
    rank[order] = np.arange(len(deg1))
    return rank


class NodeSpace:
    """Permutation/sharding info for one node type."""

    def __init__(self, n, rank, ncores):
        assert n % ncores == 0
        self.n = n
        self.nc = n // ncores            # nodes per core
        self.rank = rank                  # node id -> global sorted rank
        self.core = (rank % ncores).astype(np.int64)
        self.local = (rank // ncores).astype(np.int64)
        self.slice_rows = self.nc + 1     # +1 pad row at the tail
        self.n_tiles = math.ceil(self.nc / P)
        # global row in the concatenated (rank-major) table
        self.table_row = self.core * self.slice_rows + self.local
        self.pad_row = self.nc            # core0's pad row (any -1e30 row works)

    def perm_ids(self, c):
        """Original node ids owned by core c in local order."""
        ids = np.where(self.core == c)[0]
        return ids[np.argsort(self.local[ids])]


def _build_ell(src, dst, s_space, d_space, ncores):
    """Build per-core ELL index arrays [128, sumK] for one block.

    Returns (khat [n_tiles], cum [n_tiles+1], idx_arrays list of [128,sumK] int32).
    idx[p, cum[t]+k] = table row of (tile t, node p)'s k-th edge source.
    """
    nc_nodes = d_space.nc
    n_tiles = d_space.n_tiles
    ecore = d_space.core[dst]
    elocal = d_space.local[dst]
    srow = s_space.table_row[src]

    # counts per (core, local)
    key = ecore * nc_nodes + elocal
    counts = np.bincount(key, minlength=ncores * nc_nodes).reshape(ncores, nc_nodes)
    ctiles = np.zeros((ncores, n_tiles * P), np.int64)
    ctiles[:, :nc_nodes] = counts
    k_per_tile = ctiles.reshape(ncores, n_tiles, P).max(-1)
    khat = k_per_tile.max(0)              # uniform across cores
    cum = np.concatenate([[0], np.cumsum(khat)]).astype(np.int64)
    sumk = int(cum[-1])

    # slot index per edge
    order = np.argsort(key, kind="stable")
    key_s = key[order]
    slot = np.arange(len(key_s)) - np.searchsorted(key_s, key_s, side="left")
    srow_s = srow[order]
    ecore_s = ecore[order]
    elocal_s = elocal[order]
    tile_s = elocal_s // P
    row_s = elocal_s % P
    col_s = cum[tile_s] + slot

    idx_arrays = []
    for c in range(ncores):
        arr = np.full((P, sumk), s_space.pad_row, np.int32)
        m = ecore_s == c
        arr[row_s[m], col_s[m]] = srow_s[m].astype(np.int32)
        idx_arrays.append(arr)
    return khat.astype(np.int64), cum, idx_arrays, sumk


def _table_slices(x, space, ncores):
    """Per-core node-major slices [nc+1, F] with -1e30 pad row, plus
    per-core feature-major slices [F, nc] (zero-padded to tile multiple)."""
    f = x.shape[1]
    tabs, xts = [], []
    for c in range(ncores):
        ids = space.perm_ids(c)
        t = np.empty((space.slice_rows, f), np.float32)
        t[: space.nc] = x[ids]
        t[space.nc] = NEG_BIG
        tabs.append(t)
        xt = np.zeros((f, space.n_tiles * P), np.float32)
        xt[:, : space.nc] = x[ids].T
        xts.append(xt)
    return tabs, xts


def _prep(inputs, sizes):
    """All host-side preparation. sizes = dict(HIT, SP, EVT)."""
    HIT, SP, EVT = sizes["HIT"], sizes["SP"], sizes["EVT"]
    ep = np.asarray(inputs["edge_planar"])
    en = np.asarray(inputs["edge_nexus"])
    es = np.asarray(inputs["edge_spevt"])

    deg_b1 = np.bincount(ep[1], minlength=HIT)
    deg_b5 = np.bincount(en[0], minlength=HIT)
    hit = NodeSpace(HIT, _rank_perm2(deg_b1, deg_b5), NCORES)
    deg_b2 = np.bincount(en[1], minlength=SP)
    deg_b4 = np.bincount(es[0], minlength=SP)
    sp = NodeSpace(SP, _rank_perm2(deg_b2, deg_b4), NCORES)
    deg_b3 = np.bincount(es[1], minlength=EVT)
    evt = NodeSpace(EVT, _rank_perm(deg_b3), NCORES)

    blocks = {}
    blocks["b1"] = _build_ell(ep[0], ep[1], hit, hit, NCORES)
    blocks["b2"] = _build_ell(en[0], en[1], hit, sp, NCORES)
    blocks["b3"] = _build_ell(es[0], es[1], sp, evt, NCORES)
    blocks["b4"] = _build_ell(es[1], es[0], evt, sp, NCORES)
    blocks["b5"] = _build_ell(en[1], en[0], sp, hit, NCORES)

    x_hit = np.asarray(inputs["x_hit"], np.float32)
    x_sp = np.asarray(inputs["x_sp"], np.float32)
    x_evt = np.asarray(inputs["x_evt"], np.float32)
    hit_tabs, hit_xts = _table_slices(x_hit, hit, NCORES)
    sp_tabs, sp_xts = _table_slices(x_sp, sp, NCORES)
    evt_tabs, evt_xts = _table_slices(x_evt, evt, NCORES)
    # full (replicated) x_hit gather table for block 1
    xhit_full = np.concatenate(hit_tabs, 0)

    of = np.asarray(inputs["of"], np.float32)
    ox = np.asarray(inputs["ox"], np.float32)
    ofts, oxts = [], []
    for c in range(NCORES):
        ids = hit.perm_ids(c)
        oft = np.zeros((1, hit.n_tiles * P), np.float32)
        oft[0, : hit.nc] = of[ids, 0]
        ofts.append(oft)
        oxt = np.zeros((16, hit.n_tiles * P), np.float32)
        oxt[:, : hit.nc] = ox[ids].T
        oxts.append(oxt)

    # weights in device-friendly layouts
    prm = inputs["params"]
    W = {}
    for name in ("plane", "p2n", "n2i", "i2n", "n2p"):
        p = prm[name]
        we = np.asarray(p["We"], np.float32)        # [128,1]
        W[name] = dict(
            we_dst=np.ascontiguousarray(we[0:64, :]),                      # [64,1]
            be=float(np.asarray(p["be"]).reshape(-1)[0]),
            we_src_bc=np.ascontiguousarray(
                np.broadcast_to(we[64:128, 0][None, :], (P, 64))),         # [128,64]
            W1a=np.ascontiguousarray(np.asarray(p["W1"], np.float32)[0:64]),
            W1b=np.ascontiguousarray(np.asarray(p["W1"], np.float32)[64:128]),
            W1=np.ascontiguousarray(np.asarray(p["W1"], np.float32)),      # host model
            b1=np.ascontiguousarray(np.asarray(p["b1"], np.float32)[:, None]),
            W2=np.ascontiguousarray(np.asarray(p["W2"], np.float32)),      # [64,64]
            b2=np.ascontiguousarray(np.asarray(p["b2"], np.float32)[:, None]),
        )
    for name in ("beta", "coord"):
        p = prm[name]
        W1 = np.asarray(p["W1"], np.float32)
        b1 = np.asarray(p["b1"], np.float32)[:, None]
        W2 = np.asarray(p["W2"], np.float32)
        b2 = np.asarray(p["b2"], np.float32)[:, None]
        W3 = np.asarray(p["W3"], np.float32)
        b3 = np.asarray(p["b3"], np.float32)[:, None]
        C = np.ascontiguousarray
        fe = W1.shape[0] - 64
        W[name] = dict(
            W1x=C(W1[0:fe]), W1h=C(W1[fe:fe + 64]),
            W1=C(W1), b1a=C(b1[0:128]), b1b=C(b1[128:192]),
            W2a=C(W2[0:128]), W2b=C(W2[128:192]),
            b2a=C(b2[0:128]), b2b=C(b2[128:192]),
            W3a=C(W3[0:128]), W3b=C(W3[128:192]), b3=C(b3),
            # full copies for the host emulation
            b1=C(b1), W2=C(W2), b2=C(b2), W3=C(W3),
        )

    return dict(
        hit=hit, sp=sp, evt=evt, blocks=blocks,
        xhit_full=xhit_full, hit_xts=hit_xts,
        sp_tabs=sp_tabs, sp_xts=sp_xts,
        evt_tabs=evt_tabs, evt_xts=evt_xts,
        ofts=ofts, oxts=oxts, W=W,
    )


# ---------------------------------------------------------------- device

def _build_nc(meta, ncores):
    import concourse.bass as bass
    import concourse.bacc as bacc
    import concourse.mybir as mybir
    import concourse.tile as tile
    from concourse.masks import make_identity

    F32 = mybir.dt.float32
    I32 = mybir.dt.int32
    AF = mybir.ActivationFunctionType
    ALU = mybir.AluOpType
    AX = mybir.AxisListType

    hit, sp, evt = meta["hit"], meta["sp"], meta["evt"]
    Wm = meta["W"]

    nc = bacc.Bacc(None, target_bir_lowering=False, debug=False,
                   num_devices=ncores)

    # ---------------- I/O declarations
    def din(name, shape, dt=F32):
        return nc.dram_tensor(name, list(shape), dt, kind="ExternalInput")

    def dout(name, shape, dt=F32):
        return nc.dram_tensor(name, list(shape), dt, kind="ExternalOutput")

    xhit_full = din("xhit_full", meta["xhit_full"].shape)
    hit_xt = din("hit_xt", (64, hit.n_tiles * P))
    sp_xt = din("sp_xt", (64, sp.n_tiles * P))
    evt_xt = din("evt_xt", (64, evt.n_tiles * P))
    oft = din("oft", (1, hit.n_tiles * P))
    oxt = din("oxt", (16, hit.n_tiles * P))

    eidx = {}
    for b in ("b1", "b2", "b3", "b4", "b5"):
        sumk = meta["blocks"][b][3]
        eidx[b] = din(f"idx_{b}", (P, sumk), I32)

    wt = {}
    for name in ("plane", "p2n", "n2i", "i2n", "n2p"):
        w = Wm[name]
        wt[name] = dict(
            we_dst=din(f"{name}_wedst", w["we_dst"].shape),
            we_src_bc=din(f"{name}_wesrc", w["we_src_bc"].shape),
            W1a=din(f"{name}_W1a", w["W1a"].shape),
            W1b=din(f"{name}_W1b", w["W1b"].shape),
            b1=din(f"{name}_b1", w["b1"].shape),
            W2=din(f"{name}_W2", w["W2"].shape),
            b2=din(f"{name}_b2", w["b2"].shape),
            be=w["be"],
        )
    for name in ("beta", "coord"):
        w = Wm[name]
        wt[name] = {k: din(f"{name}_{k}", w[k].shape)
                    for k in ("W1x", "W1h", "b1a", "b1b", "W2a", "W2b", "b2a", "b2b", "W3a", "W3b", "b3")}

    h_out = dout("h_out", (64, hit.n_tiles * P))
    sp_out = dout("sp_out", (sp.nc, 64))
    evt_out = dout("evt_out", (evt.nc, 64))
    of_out = dout("of_out", (1, hit.n_tiles * P))
    ox_out = dout("ox_out", (16, hit.n_tiles * P))

    with tile.TileContext(nc) as tc:
        import contextlib
        with contextlib.ExitStack() as ctx:
            const = ctx.enter_context(tc.tile_pool(name="const", bufs=1))
            sbuf = ctx.enter_context(tc.tile_pool(name="sbuf", bufs=3))
            big = ctx.enter_context(tc.tile_pool(name="big", bufs=2))
            psum = ctx.enter_context(tc.tile_pool(name="psum", bufs=2, space="PSUM"))
            psum1 = ctx.enter_context(tc.tile_pool(name="psum1", bufs=2, space="PSUM"))
            dram = ctx.enter_context(tc.tile_pool(name="dram", bufs=1, space="DRAM"))

            ident = const.tile([P, P], F32)
            make_identity(nc, ident[:])
            zero64 = const.tile([64, P], F32)
            nc.vector.memset(zero64[:], 0.0)

            # load all weights into SBUF once
            wsb = {}
            for name in ("plane", "p2n", "n2i", "i2n", "n2p"):
                w = wt[name]
                d = {}
                for k, pp in (("we_dst", (64, 1)), ("we_src_bc", (P, 64)),
                              ("W1a", (64, 64)), ("W1b", (64, 64)),
                              ("b1", (64, 1)),
                              ("W2", (64, 64)), ("b2", (64, 1))):
                    t = const.tile(list(pp), F32, tag=f"w_{name}_{k}")
                    nc.sync.dma_start(t[:], w[k][:])
                    d[k] = t
                d["be"] = w["be"]
                wsb[name] = d
            for name in ("beta", "coord"):
                w = wt[name]
                d = {}
                for k in ("W1x", "W1h", "b1a", "b1b", "W2a", "W2b", "b2a", "b2b", "W3a", "W3b", "b3"):
                    shp = list(Wm[name][k].shape)
                    t = const.tile(shp, F32, tag=f"w_{name}_{k}")
                    nc.sync.dma_start(t[:], w[k][:])
                    d[k] = t
                wsb[name] = d

            # internal DRAM buffers
            h1_xt = dram.tile([64, hit.n_tiles * P], F32)     # h1 feature-major (local)
            sp2_xt = dram.tile([64, sp.n_tiles * P], F32)
            h1_slice = dram.tile([hit.slice_rows, 64], F32)
            sp2_slice = dram.tile([sp.slice_rows, 64], F32)
            evt3_slice = dram.tile([evt.slice_rows, 64], F32)
            sp4_slice = dram.tile([sp.slice_rows, 64], F32)
            h1_full = dram.tile([ncores * hit.slice_rows, 64], F32)
            sp2_full = dram.tile([ncores * sp.slice_rows, 64], F32)
            evt3_full = dram.tile([ncores * evt.slice_rows, 64], F32)
            sp4_full = dram.tile([ncores * sp.slice_rows, 64], F32)

            def init_pad_row(slice_buf):
                padr = sbuf.tile([1, 64], F32, tag="padr")
                nc.vector.memset(padr[:], NEG_BIG)
                nc.sync.dma_start(slice_buf[-1:, :], padr[:])

            for sl in (h1_slice, sp2_slice, evt3_slice, sp4_slice):
                init_pad_row(sl)

            def allgather(sl, full):
                nc.gpsimd.collective_compute(
                    "AllGather", ALU.bypass,
                    replica_groups=[list(range(ncores))],
                    ins=[sl.opt()], outs=[full.opt()],
                )

            def mish_from_psum(pp, bias_ap, out_tile, n_part):
                """out = mish(pp + bias); all ACT ops stay in exp_and_others.
                mish(x) = x * (1 - 2/((1+e^x)^2 + 1))"""
                xb = sbuf.tile([n_part, P], F32, tag="mish_xb")
                nc.scalar.activation(xb[:], pp[:], AF.Identity, bias=bias_ap)
                ex = sbuf.tile([n_part, P], F32, tag="mish_ex")
                nc.scalar.activation(ex[:], xb[:], AF.Exp)
                sq = sbuf.tile([n_part, P], F32, tag="mish_sq")
                nc.scalar.activation(sq[:], ex[:], AF.Square, bias=1.0)
                nc.vector.tensor_scalar_add(sq[:], sq[:], 1.0)
                nc.vector.reciprocal(sq[:], sq[:])
                nc.vector.tensor_scalar(out=sq[:], in0=sq[:], scalar1=-2.0,
                                        scalar2=1.0, op0=ALU.mult, op1=ALU.add)
                nc.vector.tensor_tensor(out=out_tile[:], in0=xb[:], in1=sq[:],
                                        op=ALU.mult)

            def sigmoid_inplace(t, n_part, width):
                """t <- sigmoid(t) using Exp table only."""
                e = sbuf.tile([n_part, width], F32, tag="sig_e")
                nc.scalar.activation(e[:], t[:], AF.Exp, scale=-1.0)
                nc.vector.tensor_scalar_add(e[:], e[:], 1.0)
                nc.vector.reciprocal(t[:], e[:])

            def mlp_block_tile(w, aggrT, xT_t):
                """2-layer block MLP, feature-major. Returns outT sbuf [64,P]."""
                ps = psum.tile([64, P], F32, tag="ps")
                nc.tensor.matmul(ps[:], lhsT=w["W1a"][:], rhs=aggrT,
                                 start=True, stop=False)
                nc.tensor.matmul(ps[:], lhsT=w["W1b"][:], rhs=xT_t,
                                 start=False, stop=True)
                h1t = sbuf.tile([64, P], F32, tag="h1t")
                mish_from_psum(ps, w["b1"][:], h1t, 64)
                ps2 = psum.tile([64, P], F32, tag="ps")
                nc.tensor.matmul(ps2[:], lhsT=w["W2"][:], rhs=h1t[:],
                                 start=True, stop=True)
                outT = sbuf.tile([64, P], F32, tag="outT")
                mish_from_psum(ps2, w["b2"][:], outT, 64)
                return outT

            def gather_block(bname, wname, table_ap, xt_ap, space, epilogue):
                """One message-passing block over this core's target tiles."""
                khat, cum, _, sumk = meta["blocks"][bname]
                w = wsb[wname]
                n_tiles = space.n_tiles
                for t in range(n_tiles):
                    K = int(khat[t])
                    c0 = int(cum[t])
                    xT_t = sbuf.tile([64, P], F32, tag="xTt")
                    nc.sync.dma_start(xT_t[:], xt_ap[:, t * P:(t + 1) * P])
                    if K > 0:
                        idx_t = sbuf.tile([P, K], I32, tag="idxt")
                        nc.sync.dma_start(idx_t[:], eidx[bname][:, c0:c0 + K])
                        # a = xT.T @ we_dst (+be later)   [P,1]
                        ps_a = psum1.tile([P, 1], F32, tag="ps1")
                        nc.tensor.matmul(ps_a[:], lhsT=xT_t[:], rhs=w["we_dst"][:],
                                         start=True, stop=True)
                        a_sb = sbuf.tile([P, 1], F32, tag="asb")
                        nc.scalar.copy(a_sb[:], ps_a[:])
                        denom = sbuf.tile([P, 64], F32, tag="denom")
                        num = sbuf.tile([P, 64], F32, tag="num")
                        for p0 in range(0, K, KCAP):
                            Kp = min(KCAP, K - p0)
                            S = big.tile([P, Kp * 64], F32, tag="S")
                            E = big.tile([P, Kp * 64], F32, tag="E")
                            for k in range(Kp):
                                nc.gpsimd.indirect_dma_start(
                                    out=S[:, k * 64:(k + 1) * 64],
                                    out_offset=None,
                                    in_=table_ap[:],
                                    in_offset=bass.IndirectOffsetOnAxis(
                                        ap=idx_t[:, p0 + k:p0 + k + 1], axis=0),
                                )
                            S3 = S[:].rearrange("p (k f) -> p k f", k=Kp)
                            E3 = E[:].rearrange("p (k f) -> p k f", k=Kp)
                            wes_view = w["we_src_bc"][:].unsqueeze(1).to_broadcast(
                                [P, Kp, 64])
                            nc.vector.tensor_tensor(out=E3, in0=S3, in1=wes_view,
                                                    op=ALU.mult)
                            bv = sbuf.tile([P, Kp], F32, tag="bv")
                            nc.vector.tensor_reduce(out=bv[:], in_=E3, axis=AX.X,
                                                    op=ALU.add)
                            logit = sbuf.tile([P, Kp], F32, tag="logit")
                            nc.scalar.activation(logit[:], bv[:], AF.Identity,
                                                 bias=a_sb[:])
                            nc.vector.tensor_scalar(out=logit[:], in0=logit[:],
                                                    scalar1=float(w["be"]),
                                                    scalar2=CLAMP_LO,
                                                    op0=ALU.add, op1=ALU.max)
                            gate = sbuf.tile([P, Kp], F32, tag="gate")
                            nc.vector.tensor_copy(gate[:], logit[:])
                            sigmoid_inplace(gate, P, Kp)
                            g_view = gate[:].unsqueeze(2).to_broadcast([P, Kp, 64])
                            nc.vector.tensor_tensor(out=S3, in0=S3, in1=g_view,
                                                    op=ALU.mult)
                            nc.scalar.activation(E[:], S[:], AF.Exp)
                            nc.vector.tensor_tensor(out=S[:], in0=S[:], in1=E[:],
                                                    op=ALU.mult)

                            def slotred(src_tile, acc, first):
                                view = src_tile[:].rearrange("p (k f) -> p f k", k=Kp)
                                if first:
                                    nc.vector.tensor_reduce(out=acc[:], in_=view,
                                                            axis=AX.X, op=ALU.add)
                                else:
                                    tmp = sbuf.tile([P, 64], F32, tag="redtmp")
                                    nc.vector.tensor_reduce(out=tmp[:], in_=view,
                                                            axis=AX.X, op=ALU.add)
                                    nc.vector.tensor_add(acc[:], acc[:], tmp[:])
                            slotred(E, denom, p0 == 0)
                            slotred(S, num, p0 == 0)
                        nc.vector.tensor_scalar_add(denom[:], denom[:], DENOM_EPS)
                        recip = sbuf.tile([P, 64], F32, tag="recip")
                        nc.vector.reciprocal(recip[:], denom[:])
                        aggr = sbuf.tile([P, 64], F32, tag="aggr")
                        nc.vector.tensor_tensor(out=aggr[:], in0=num[:],
                                                in1=recip[:], op=ALU.mult)
                        # transpose aggr -> [64, P]
                        ps_t = psum.tile([64, P], F32, tag="ps")
                        nc.tensor.transpose(ps_t[:], aggr[:], ident[:])
                        aggrT = sbuf.tile([64, P], F32, tag="aggrTs")
                        nc.scalar.copy(aggrT[:], ps_t[:])
                        aggrT_ap = aggrT[:]
                    else:
                        aggrT_ap = zero64[:]
                    outT = mlp_block_tile(w, aggrT_ap, xT_t[:])
                    epilogue(t, outT, xT_t)

            def write_nodemajor(t, outT, space, slice_buf, ext_out=None):
                """Transpose outT back to node-major, write slice rows."""
                ps_b = psum.tile([P, 64], F32, tag="ps")
                nc.tensor.transpose(ps_b[:], outT[:], ident[0:64, 0:64])
                nm = sbuf.tile([P, 64], F32, tag="nms")
                nc.scalar.copy(nm[:], ps_b[:])
                lo = t * P
                hi = min(space.nc, lo + P)
                if hi > lo:
                    nc.sync.dma_start(slice_buf[lo:hi, :], nm[0:hi - lo, :])
                    if ext_out is not None:
                        nc.sync.dma_start(ext_out[lo:hi, :], nm[0:hi - lo, :])

            # ---------------- block 1: plane (hit<-hit), src table = xhit_full
            def epi_b1(t, outT, xT_t):
                nc.sync.dma_start(h1_xt[:, t * P:(t + 1) * P], outT[:])
                write_nodemajor(t, outT, hit, h1_slice)

            gather_block("b1", "plane", xhit_full, hit_xt, hit, epi_b1)
            allgather(h1_slice, h1_full)

            # ---------------- block 2: p2n (sp <- h1), dst x = x_sp
            def epi_b2(t, outT, xT_t):
                nc.sync.dma_start(sp2_xt[:, t * P:(t + 1) * P], outT[:])
                write_nodemajor(t, outT, sp, sp2_slice)

            gather_block("b2", "p2n", h1_full, sp_xt, sp, epi_b2)
            allgather(sp2_slice, sp2_full)

            # ---------------- block 3: n2i (evt <- sp2), dst x = x_evt
            def epi_b3(t, outT, xT_t):
                write_nodemajor(t, outT, evt, evt3_slice, ext_out=evt_out)

            gather_block("b3", "n2i", sp2_full, evt_xt, evt, epi_b3)
            allgather(evt3_slice, evt3_full)

            # ---------------- block 4: i2n (sp <- evt3), dst x = sp2
            def epi_b4(t, outT, xT_t):
                write_nodemajor(t, outT, sp, sp4_slice, ext_out=sp_out)

            gather_block("b4", "i2n", evt3_full, sp2_xt, sp, epi_b4)
            allgather(sp4_slice, sp4_full)

            # ---------------- block 5: n2p (hit <- sp4), dst x = h1; + final MLPs
            def epi_b5(t, outT, xT_t):
                # outT = h5 tile [64, P]; xT_t = h1 tile
                nc.sync.dma_start(h_out[:, t * P:(t + 1) * P], outT[:])
                # ---- beta MLP: cat [of(1) | h(64)] -> 192 -> 192 -> 1 sigmoid
                for name, cat_extra, extra_ap, out_ext, final in (
                    ("beta", 1, oft, of_out, "sigmoid"),
                    ("coord", 16, oxt, ox_out, "none"),
                ):
                    w = wsb[name]
                    fe = cat_extra
                    ex = sbuf.tile([fe, P], F32, tag=f"ex{name}")
                    nc.sync.dma_start(ex[:], extra_ap[:, t * P:(t + 1) * P])
                    # layer 1: [fe+64] -> 192, M-split 128+64
                    h1a = sbuf.tile([P, P], F32, tag=f"{name}h1a")
                    h1b = sbuf.tile([64, P], F32, tag=f"{name}h1b")
                    for (mlo, mhi, ht, b1t) in ((0, 128, h1a, w["b1a"]),
                                                (128, 192, h1b, w["b1b"])):
                        pp = psum.tile([mhi - mlo, P], F32, tag="ps")
                        nc.tensor.matmul(pp[:], lhsT=w["W1x"][:, mlo:mhi],
                                         rhs=ex[:], start=True, stop=False)
                        nc.tensor.matmul(pp[:], lhsT=w["W1h"][:, mlo:mhi],
                                         rhs=outT[:], start=False, stop=True)
                        mish_from_psum(pp, b1t[:], ht, mhi - mlo)
                    # layer 2: 192 -> 192
                    h2a = sbuf.tile([P, P], F32, tag=f"{name}h2a")
                    h2b = sbuf.tile([64, P], F32, tag=f"{name}h2b")
                    for (mlo, mhi, ht, b2t) in ((0, 128, h2a, w["b2a"]),
                                                (128, 192, h2b, w["b2b"])):
                        pp = psum.tile([mhi - mlo, P], F32, tag="ps")
                        nc.tensor.matmul(pp[:], lhsT=w["W2a"][:, mlo:mhi],
                                         rhs=h1a[:], start=True, stop=False)
                        nc.tensor.matmul(pp[:], lhsT=w["W2b"][:, mlo:mhi],
                                         rhs=h1b[:], start=False, stop=True)
                        mish_from_psum(pp, b2t[:], ht, mhi - mlo)
                    # layer 3: 192 -> fo
                    fo = Wm[name]["W3a"].shape[1]
                    pp = psum1.tile([fo, P], F32, tag="ps1")
                    nc.tensor.matmul(pp[:], lhsT=w["W3a"][:], rhs=h2a[:],
                                     start=True, stop=False)
                    nc.tensor.matmul(pp[:], lhsT=w["W3b"][:], rhs=h2b[:],
                                     start=False, stop=True)
                    ot = sbuf.tile([fo, P], F32, tag=f"{name}out")
                    nc.scalar.activation(ot[:], pp[:], AF.Identity,
                                         bias=w["b3"][:])
                    if final == "sigmoid":
                        sigmoid_inplace(ot, fo, P)
                    nc.sync.dma_start(out_ext[:, t * P:(t + 1) * P], ot[:])

            gather_block("b5", "n2p", sp4_full, h1_xt, hit, epi_b5)

    nc.finalize()
    return nc


# ---------------------------------------------------------------- runner

def _in_maps(meta, ncores):
    maps = []
    for c in range(ncores):
        m = dict(
            xhit_full=meta["xhit_full"],
            hit_xt=meta["hit_xts"][c],
            sp_xt=meta["sp_xts"][c],
            evt_xt=meta["evt_xts"][c],
            oft=meta["ofts"][c],
            oxt=meta["oxts"][c],
        )
        for b in ("b1", "b2", "b3", "b4", "b5"):
            m[f"idx_{b}"] = meta["blocks"][b][2][c]
        Wm = meta["W"]
        for name in ("plane", "p2n", "n2i", "i2n", "n2p"):
            w = Wm[name]
            m[f"{name}_wedst"] = w["we_dst"]
            m[f"{name}_wesrc"] = w["we_src_bc"]
            for k in ("W1a", "W1b", "b1", "W2", "b2"):
                m[f"{name}_{k}"] = w[k]
        for name in ("beta", "coord"):
            for k in ("W1x", "W1h", "b1a", "b1b", "W2a", "W2b", "b2a", "b2b", "W3a", "W3b", "b3"):
                m[f"{name}_{k}"] = Wm[name][k]
        maps.append(m)
    return maps


def _postprocess(meta, results, sizes):
    hit, sp, evt = meta["hit"], meta["sp"], meta["evt"]
    HIT, SP, EVT = sizes["HIT"], sizes["SP"], sizes["EVT"]
    h = np.empty((HIT, 64), np.float32)
    of_new = np.empty((HIT, 1), np.float32)
    ox_new = np.empty((HIT, 16), np.float32)
    sp_new = np.empty((SP, 64), np.float32)
    evt_new = np.empty((EVT, 64), np.float32)
    for c in range(NCORES):
        r = results[c]
        ids = hit.perm_ids(c)
        h[ids] = r["h_out"][:, : hit.nc].T
        of_new[ids, 0] = r["of_out"][0, : hit.nc]
        ox_new[ids] = r["ox_out"][:, : hit.nc].T
        sp_new[sp.perm_ids(c)] = r["sp_out"]
        evt_new[evt.perm_ids(c)] = r["evt_out"]
    return h, sp_new, evt_new, of_new, ox_new


def run_model(inputs, sizes, ncores=NCORES, use_sim=False):
    """Build (cached), run, and postprocess."""
    meta = _prep(inputs, sizes)
    key = ("nc", tuple(sorted(sizes.items())), ncores,
           tuple(int(meta["blocks"][b][3]) for b in ("b1", "b2", "b3", "b4", "b5")))
    if key not in _CACHE:
        _CACHE[key] = _build_nc(meta, ncores)
    nc = _CACHE[key]
    in_maps = _in_maps(meta, ncores)

    if use_sim:
        from concourse.bass_interp import MultiCoreSim
        sim = MultiCoreSim(nc, num_cores=ncores, require_finite=False,
                           require_nnan=False)
        sims = list(sim.cores.values())
        for c, cs in enumerate(sims):
            for k, v in in_maps[c].items():
                cs.tensor(k)[:] = v
        sim.simulate(check_with_hw=False)
        outs = ("h_out", "sp_out", "evt_out", "of_out", "ox_out")
        results = [{k: np.asarray(cs.tensor(k)) for k in outs} for cs in sims]
    else:
        rkey = ("runner", key)
        if rkey not in _CACHE:
            from runner import SpmdRunner
            _CACHE[rkey] = SpmdRunner(nc, ncores)
        r = _CACHE[rkey]
        r.put(in_maps)
        results = r.results(r.run())
    return _postprocess(meta, results, sizes)


def kernel(**inputs):
    sizes = dict(HIT=200000, SP=40000, EVT=512)
    return run_model(inputs, sizes)


# revision 14
# speedup vs baseline: 1.1910x; 1.1910x over previous
"""NuGraphCore GNN message passing on 8 trn2 NeuronCores (Bass/Tile).

Strategy (target-sharded ELL):
 - Nodes of each type (hit/sp/evt) are degree-sorted and round-robin
   assigned to 8 cores; each core owns the edges whose TARGET it owns.
 - Per block, each core processes its target tiles (128 nodes) in ELL
   layout: K_t slot columns per tile, each slot gathered from the full
   (replicated / allgathered) source table via int32 indirect DMA.
 - Softmax aggregation is done without segment-max (ratio-invariant):
   e = exp(gate*xj), aggr = sum(m*e)/sum(e), pads use -1e30 rows -> e=0.
 - Block MLPs run feature-major on PE; outputs are AllGathered between
   blocks as gather sources for the next block.
"""
import math
import numpy as np

NCORES = 8
P = 128
NEG_BIG = -1.0e30
CLAMP_LO = -55.0
KCAP = 16
DENOM_EPS = 1e-30

_CACHE = {}


# ---------------------------------------------------------------- host prep

def _rank_perm(degrees):
    """Sort nodes by (deg desc) stably; return rank per node id."""
    order = np.lexsort((np.arange(len(degrees)), -degrees))
    rank = np.empty(len(degrees), np.int64)
    rank[order] = np.arange(len(degrees))
    return rank


def _rank_perm2(deg1, deg2):
    order = np.lexsort((np.arange(len(deg1)), -deg2, -deg1))
    rank = np.empty(len(deg1), np.int64)
    rank[order] = np.arange(len(deg1))
    return rank


class NodeSpace:
    """Permutation/sharding info for one node type."""

    def __init__(self, n, rank, ncores):
        assert n % ncores == 0
        self.n = n
        self.nc = n // ncores            # nodes per core
        self.rank = rank                  # node id -> global sorted rank
        self.core = (rank % ncores).astype(np.int64)
        self.local = (rank // ncores).astype(np.int64)
        self.slice_rows = self.nc + 1     # +1 pad row at the tail
        self.n_tiles = math.ceil(self.nc / P)
        # global row in the concatenated (rank-major) table
        self.table_row = self.core * self.slice_rows + self.local
        self.pad_row = self.nc            # core0's pad row (any -1e30 row works)

    def perm_ids(self, c):
        """Original node ids owned by core c in local order."""
        ids = np.where(self.core == c)[0]
        return ids[np.argsort(self.local[ids])]


def _build_ell(src, dst, s_space, d_space, ncores):
    """Build per-core ELL index arrays [128, sumK] for one block.

    Returns (khat [n_tiles], cum [n_tiles+1], idx_arrays list of [128,sumK] int32).
    idx[p, cum[t]+k] = table row of (tile t, node p)'s k-th edge source.
    """
    nc_nodes = d_space.nc
    n_tiles = d_space.n_tiles
    ecore = d_space.core[dst]
    elocal = d_space.local[dst]
    srow = s_space.table_row[src]

    # counts per (core, local)
    key = ecore * nc_nodes + elocal
    counts = np.bincount(key, minlength=ncores * nc_nodes).reshape(ncores, nc_nodes)
    ctiles = np.zeros((ncores, n_tiles * P), np.int64)
    ctiles[:, :nc_nodes] = counts
    k_per_tile = ctiles.reshape(ncores, n_tiles, P).max(-1)
    khat = k_per_tile.max(0)              # uniform across cores
    cum = np.concatenate([[0], np.cumsum(khat)]).astype(np.int64)
    sumk = int(cum[-1])

    # slot index per edge
    order = np.argsort(key, kind="stable")
    key_s = key[order]
    slot = np.arange(len(key_s)) - np.searchsorted(key_s, key_s, side="left")
    srow_s = srow[order]
    ecore_s = ecore[order]
    elocal_s = elocal[order]
    tile_s = elocal_s // P
    row_s = elocal_s % P
    col_s = cum[tile_s] + slot

    idx_arrays = []
    for c in range(ncores):
        arr = np.full((P, sumk), s_space.pad_row, np.int32)
        m = ecore_s == c
        arr[row_s[m], col_s[m]] = srow_s[m].astype(np.int32)
        idx_arrays.append(arr)
    return khat.astype(np.int64), cum, idx_arrays, sumk


def _table_slices(x, space, ncores):
    """Per-core node-major slices [nc+1, F] with -1e30 pad row, plus
    per-core feature-major slices [F, nc] (zero-padded to tile multiple)."""
    f = x.shape[1]
    tabs, xts = [], []
    for c in range(ncores):
        ids = space.perm_ids(c)
        t = np.empty((space.slice_rows, f), np.float32)
        t[: space.nc] = x[ids]
        t[space.nc] = NEG_BIG
        tabs.append(t)
        xt = np.zeros((f, space.n_tiles * P), np.float32)
        xt[:, : space.nc] = x[ids].T
        xts.append(xt)
    return tabs, xts


def _prep(inputs, sizes):
    """All host-side preparation. sizes = dict(HIT, SP, EVT)."""
    HIT, SP, EVT = sizes["HIT"], sizes["SP"], sizes["EVT"]
    ep = np.asarray(inputs["edge_planar"])
    en = np.asarray(inputs["edge_nexus"])
    es = np.asarray(inputs["edge_spevt"])

    deg_b1 = np.bincount(ep[1], minlength=HIT)
    deg_b5 = np.bincount(en[0], minlength=HIT)
    hit = NodeSpace(HIT, _rank_perm2(deg_b1, deg_b5), NCORES)
    deg_b2 = np.bincount(en[1], minlength=SP)
    deg_b4 = np.bincount(es[0], minlength=SP)
    sp = NodeSpace(SP, _rank_perm2(deg_b2, deg_b4), NCORES)
    deg_b3 = np.bincount(es[1], minlength=EVT)
    evt = NodeSpace(EVT, _rank_perm(deg_b3), NCORES)

    blocks = {}
    blocks["b1"] = _build_ell(ep[0], ep[1], hit, hit, NCORES)
    blocks["b2"] = _build_ell(en[0], en[1], hit, sp, NCORES)
    blocks["b3"] = _build_ell(es[0], es[1], sp, evt, NCORES)
    blocks["b4"] = _build_ell(es[1], es[0], evt, sp, NCORES)
    blocks["b5"] = _build_ell(en[1], en[0], sp, hit, NCORES)

    x_hit = np.asarray(inputs["x_hit"], np.float32)
    x_sp = np.asarray(inputs["x_sp"], np.float32)
    x_evt = np.asarray(inputs["x_evt"], np.float32)
    hit_tabs, hit_xts = _table_slices(x_hit, hit, NCORES)
    sp_tabs, sp_xts = _table_slices(x_sp, sp, NCORES)
    evt_tabs, evt_xts = _table_slices(x_evt, evt, NCORES)
    # full (replicated) x_hit gather table for block 1
    xhit_full = np.concatenate(hit_tabs, 0)

    of = np.asarray(inputs["of"], np.float32)
    ox = np.asarray(inputs["ox"], np.float32)
    ofts, oxts = [], []
    for c in range(NCORES):
        ids = hit.perm_ids(c)
        oft = np.zeros((1, hit.n_tiles * P), np.float32)
        oft[0, : hit.nc] = of[ids, 0]
        ofts.append(oft)
        oxt = np.zeros((16, hit.n_tiles * P), np.float32)
        oxt[:, : hit.nc] = ox[ids].T
        oxts.append(oxt)

    # weights in device-friendly layouts
    prm = inputs["params"]
    W = {}
    for name in ("plane", "p2n", "n2i", "i2n", "n2p"):
        p = prm[name]
        we = np.asarray(p["We"], np.float32)        # [128,1]
        W[name] = dict(
            we_dst=np.ascontiguousarray(we[0:64, :]),                      # [64,1]
            be=float(np.asarray(p["be"]).reshape(-1)[0]),
            we_src_bc=np.ascontiguousarray(
                np.broadcast_to(we[64:128, 0][None, :], (P, 64))),         # [128,64]
            W1a=np.ascontiguousarray(np.asarray(p["W1"], np.float32)[0:64]),
            W1b=np.ascontiguousarray(np.asarray(p["W1"], np.float32)[64:128]),
            W1=np.ascontiguousarray(np.asarray(p["W1"], np.float32)),      # host model
            b1=np.ascontiguousarray(np.asarray(p["b1"], np.float32)[:, None]),
            W2=np.ascontiguousarray(np.asarray(p["W2"], np.float32)),      # [64,64]
            b2=np.ascontiguousarray(np.asarray(p["b2"], np.float32)[:, None]),
        )
    for name in ("beta", "coord"):
        p = prm[name]
        W1 = np.asarray(p["W1"], np.float32)
        b1 = np.asarray(p["b1"], np.float32)[:, None]
        W2 = np.asarray(p["W2"], np.float32)
        b2 = np.asarray(p["b2"], np.float32)[:, None]
        W3 = np.asarray(p["W3"], np.float32)
        b3 = np.asarray(p["b3"], np.float32)[:, None]
        C = np.ascontiguousarray
        fe = W1.shape[0] - 64
        W[name] = dict(
            W1x=C(W1[0:fe]), W1h=C(W1[fe:fe + 64]),
            W1=C(W1), b1a=C(b1[0:128]), b1b=C(b1[128:192]),
            W2a=C(W2[0:128]), W2b=C(W2[128:192]),
            b2a=C(b2[0:128]), b2b=C(b2[128:192]),
            W3a=C(W3[0:128]), W3b=C(W3[128:192]), b3=C(b3),
            # full copies for the host emulation
            b1=C(b1), W2=C(W2), b2=C(b2), W3=C(W3),
        )

    return dict(
        hit=hit, sp=sp, evt=evt, blocks=blocks,
        xhit_full=xhit_full, hit_xts=hit_xts,
        sp_tabs=sp_tabs, sp_xts=sp_xts,
        evt_tabs=evt_tabs, evt_xts=evt_xts,
        ofts=ofts, oxts=oxts, W=W,
    )


# ---------------------------------------------------------------- device

def _build_nc(meta, ncores):
    import concourse.bass as bass
    import concourse.bacc as bacc
    import concourse.mybir as mybir
    import concourse.tile as tile
    from concourse.masks import make_identity

    F32 = mybir.dt.float32
    I32 = mybir.dt.int32
    AF = mybir.ActivationFunctionType
    ALU = mybir.AluOpType
    AX = mybir.AxisListType

    hit, sp, evt = meta["hit"], meta["sp"], meta["evt"]
    Wm = meta["W"]

    nc = bacc.Bacc(None, target_bir_lowering=False, debug=False,
                   num_devices=ncores)

    # ---------------- I/O declarations
    def din(name, shape, dt=F32):
        return nc.dram_tensor(name, list(shape), dt, kind="ExternalInput")

    def dout(name, shape, dt=F32):
        return nc.dram_tensor(name, list(shape), dt, kind="ExternalOutput")

    xhit_full = din("xhit_full", meta["xhit_full"].shape)
    hit_xt = din("hit_xt", (64, hit.n_tiles * P))
    sp_xt = din("sp_xt", (64, sp.n_tiles * P))
    evt_xt = din("evt_xt", (64, evt.n_tiles * P))
    oft = din("oft", (1, hit.n_tiles * P))
    oxt = din("oxt", (16, hit.n_tiles * P))

    eidx = {}
    for b in ("b1", "b2", "b3", "b4", "b5"):
        sumk = meta["blocks"][b][3]
        eidx[b] = din(f"idx_{b}", (P, sumk), I32)

    wt = {}
    for name in ("plane", "p2n", "n2i", "i2n", "n2p"):
        w = Wm[name]
        wt[name] = dict(
            we_dst=din(f"{name}_wedst", w["we_dst"].shape),
            we_src_bc=din(f"{name}_wesrc", w["we_src_bc"].shape),
            W1a=din(f"{name}_W1a", w["W1a"].shape),
            W1b=din(f"{name}_W1b", w["W1b"].shape),
            b1=din(f"{name}_b1", w["b1"].shape),
            W2=din(f"{name}_W2", w["W2"].shape),
            b2=din(f"{name}_b2", w["b2"].shape),
            be=w["be"],
        )
    for name in ("beta", "coord"):
        w = Wm[name]
        wt[name] = {k: din(f"{name}_{k}", w[k].shape)
                    for k in ("W1x", "W1h", "b1a", "b1b", "W2a", "W2b", "b2a", "b2b", "W3a", "W3b", "b3")}

    h_out = dout("h_out", (64, hit.n_tiles * P))
    sp_out = dout("sp_out", (sp.nc, 64))
    evt_out = dout("evt_out", (evt.nc, 64))
    of_out = dout("of_out", (1, hit.n_tiles * P))
    ox_out = dout("ox_out", (16, hit.n_tiles * P))

    with tile.TileContext(nc) as tc:
        import contextlib
        with contextlib.ExitStack() as ctx:
            const = ctx.enter_context(tc.tile_pool(name="const", bufs=1))
            sbuf = ctx.enter_context(tc.tile_pool(name="sbuf", bufs=4))
            big = ctx.enter_context(tc.tile_pool(name="big", bufs=4))
            psum = ctx.enter_context(tc.tile_pool(name="psum", bufs=3, space="PSUM"))
            psum1 = ctx.enter_context(tc.tile_pool(name="psum1", bufs=3, space="PSUM"))
            dram = ctx.enter_context(tc.tile_pool(name="dram", bufs=1, space="DRAM"))

            ident = const.tile([P, P], F32)
            make_identity(nc, ident[:])
            zero64 = const.tile([64, P], F32)
            nc.vector.memset(zero64[:], 0.0)

            # load all weights into SBUF once
            wsb = {}
            for name in ("plane", "p2n", "n2i", "i2n", "n2p"):
                w = wt[name]
                d = {}
                for k, pp in (("we_dst", (64, 1)), ("we_src_bc", (P, 64)),
                              ("W1a", (64, 64)), ("W1b", (64, 64)),
                              ("b1", (64, 1)),
                              ("W2", (64, 64)), ("b2", (64, 1))):
                    t = const.tile(list(pp), F32, tag=f"w_{name}_{k}")
                    nc.sync.dma_start(t[:], w[k][:])
                    d[k] = t
                d["be"] = w["be"]
                wsb[name] = d
            for name in ("beta", "coord"):
                w = wt[name]
                d = {}
                for k in ("W1x", "W1h", "b1a", "b1b", "W2a", "W2b", "b2a", "b2b", "W3a", "W3b", "b3"):
                    shp = list(Wm[name][k].shape)
                    t = const.tile(shp, F32, tag=f"w_{name}_{k}")
                    nc.sync.dma_start(t[:], w[k][:])
                    d[k] = t
                wsb[name] = d

            # internal DRAM buffers
            h1_xt = dram.tile([64, hit.n_tiles * P], F32)     # h1 feature-major (local)
            sp2_xt = dram.tile([64, sp.n_tiles * P], F32)
            h1_slice = dram.tile([hit.slice_rows, 64], F32)
            sp2_slice = dram.tile([sp.slice_rows, 64], F32)
            evt3_slice = dram.tile([evt.slice_rows, 64], F32)
            sp4_slice = dram.tile([sp.slice_rows, 64], F32)
            h1_full = dram.tile([ncores * hit.slice_rows, 64], F32)
            sp2_full = dram.tile([ncores * sp.slice_rows, 64], F32)
            evt3_full = dram.tile([ncores * evt.slice_rows, 64], F32)
            sp4_full = dram.tile([ncores * sp.slice_rows, 64], F32)

            def init_pad_row(slice_buf):
                padr = sbuf.tile([1, 64], F32, tag="padr")
                nc.vector.memset(padr[:], NEG_BIG)
                nc.sync.dma_start(slice_buf[-1:, :], padr[:])

            for sl in (h1_slice, sp2_slice, evt3_slice, sp4_slice):
                init_pad_row(sl)

            def allgather(sl, full):
                nc.gpsimd.collective_compute(
                    "AllGather", ALU.bypass,
                    replica_groups=[list(range(ncores))],
                    ins=[sl.opt()], outs=[full.opt()],
                )

            def mish_from_psum(pp, bias_ap, out_tile, n_part):
                """out = mish(pp + bias); all ACT ops stay in exp_and_others.
                mish(x) = x * (1 - 2/((1+e^x)^2 + 1))"""
                xb = sbuf.tile([n_part, P], F32, tag="mish_xb")
                nc.scalar.activation(xb[:], pp[:], AF.Identity, bias=bias_ap)
                ex = sbuf.tile([n_part, P], F32, tag="mish_ex")
                nc.scalar.activation(ex[:], xb[:], AF.Exp)
                sq = sbuf.tile([n_part, P], F32, tag="mish_sq")
                nc.scalar.activation(sq[:], ex[:], AF.Square, bias=1.0)
                nc.vector.tensor_scalar_add(sq[:], sq[:], 1.0)
                nc.vector.reciprocal(sq[:], sq[:])
                nc.vector.tensor_scalar(out=sq[:], in0=sq[:], scalar1=-2.0,
                                        scalar2=1.0, op0=ALU.mult, op1=ALU.add)
                nc.vector.tensor_tensor(out=out_tile[:], in0=xb[:], in1=sq[:],
                                        op=ALU.mult)

            def sigmoid_inplace(t, n_part, width):
                """t <- sigmoid(t) using Exp table only."""
                e = sbuf.tile([n_part, width], F32, tag="sig_e")
                nc.scalar.activation(e[:], t[:], AF.Exp, scale=-1.0)
                nc.vector.tensor_scalar_add(e[:], e[:], 1.0)
                nc.vector.reciprocal(t[:], e[:])

            def mlp_block_tile(w, aggrT, xT_t):
                """2-layer block MLP, feature-major. Returns outT sbuf [64,P]."""
                ps = psum.tile([64, P], F32, tag="ps")
                nc.tensor.matmul(ps[:], lhsT=w["W1a"][:], rhs=aggrT,
                                 start=True, stop=False)
                nc.tensor.matmul(ps[:], lhsT=w["W1b"][:], rhs=xT_t,
                                 start=False, stop=True)
                h1t = sbuf.tile([64, P], F32, tag="h1t")
                mish_from_psum(ps, w["b1"][:], h1t, 64)
                ps2 = psum.tile([64, P], F32, tag="ps")
                nc.tensor.matmul(ps2[:], lhsT=w["W2"][:], rhs=h1t[:],
                                 start=True, stop=True)
                outT = sbuf.tile([64, P], F32, tag="outT")
                mish_from_psum(ps2, w["b2"][:], outT, 64)
                return outT

            def gather_block(bname, wname, table_ap, xt_ap, space, epilogue):
                """One message-passing block over this core's target tiles."""
                khat, cum, _, sumk = meta["blocks"][bname]
                w = wsb[wname]
                n_tiles = space.n_tiles
                for t in range(n_tiles):
                    K = int(khat[t])
                    c0 = int(cum[t])
                    xT_t = sbuf.tile([64, P], F32, tag="xTt")
                    nc.sync.dma_start(xT_t[:], xt_ap[:, t * P:(t + 1) * P])
                    if K > 0:
                        idx_t = sbuf.tile([P, K], I32, tag="idxt")
                        nc.sync.dma_start(idx_t[:], eidx[bname][:, c0:c0 + K])
                        # a = xT.T @ we_dst (+be later)   [P,1]
                        ps_a = psum1.tile([P, 1], F32, tag="ps1")
                        nc.tensor.matmul(ps_a[:], lhsT=xT_t[:], rhs=w["we_dst"][:],
                                         start=True, stop=True)
                        a_sb = sbuf.tile([P, 1], F32, tag="asb")
                        nc.scalar.copy(a_sb[:], ps_a[:])
                        denom = sbuf.tile([P, 64], F32, tag="denom")
                        num = sbuf.tile([P, 64], F32, tag="num")
                        for p0 in range(0, K, KCAP):
                            Kp = min(KCAP, K - p0)
                            S = big.tile([P, Kp * 64], F32, tag="S")
                            E = big.tile([P, Kp * 64], F32, tag="E")
                            for k in range(Kp):
                                nc.gpsimd.indirect_dma_start(
                                    out=S[:, k * 64:(k + 1) * 64],
                                    out_offset=None,
                                    in_=table_ap[:],
                                    in_offset=bass.IndirectOffsetOnAxis(
                                        ap=idx_t[:, p0 + k:p0 + k + 1], axis=0),
                                )
                            S3 = S[:].rearrange("p (k f) -> p k f", k=Kp)
                            E3 = E[:].rearrange("p (k f) -> p k f", k=Kp)
                            wes_view = w["we_src_bc"][:].unsqueeze(1).to_broadcast(
                                [P, Kp, 64])
                            nc.vector.tensor_tensor(out=E3, in0=S3, in1=wes_view,
                                                    op=ALU.mult)
                            bv = sbuf.tile([P, Kp], F32, tag="bv")
                            nc.vector.tensor_reduce(out=bv[:], in_=E3, axis=AX.X,
                                                    op=ALU.add)
                            logit = sbuf.tile([P, Kp], F32, tag="logit")
                            nc.scalar.activation(logit[:], bv[:], AF.Identity,
                                                 bias=a_sb[:])
                            nc.vector.tensor_scalar(out=logit[:], in0=logit[:],
                                                    scalar1=float(w["be"]),
                                                    scalar2=CLAMP_LO,
                                                    op0=ALU.add, op1=ALU.max)
                            gate = sbuf.tile([P, Kp], F32, tag="gate")
                            nc.vector.tensor_copy(gate[:], logit[:])
                            sigmoid_inplace(gate, P, Kp)
                            g_view = gate[:].unsqueeze(2).to_broadcast([P, Kp, 64])
                            nc.vector.tensor_tensor(out=S3, in0=S3, in1=g_view,
                                                    op=ALU.mult)
                            nc.scalar.activation(E[:], S[:], AF.Exp)
                            nc.vector.tensor_tensor(out=S[:], in0=S[:], in1=E[:],
                                                    op=ALU.mult)

                            def slotred(src_tile, acc, first):
                                view = src_tile[:].rearrange("p (k f) -> p f k", k=Kp)
                                if first:
                                    nc.vector.tensor_reduce(out=acc[:], in_=view,
                                                            axis=AX.X, op=ALU.add)
                                else:
                                    tmp = sbuf.tile([P, 64], F32, tag="redtmp")
                                    nc.vector.tensor_reduce(out=tmp[:], in_=view,
                                                            axis=AX.X, op=ALU.add)
                                    nc.vector.tensor_add(acc[:], acc[:], tmp[:])
                            slotred(E, denom, p0 == 0)
                            slotred(S, num, p0 == 0)
                        nc.vector.tensor_scalar_add(denom[:], denom[:], DENOM_EPS)
                        recip = sbuf.tile([P, 64], F32, tag="recip")
                        nc.vector.reciprocal(recip[:], denom[:])
                        aggr = sbuf.tile([P, 64], F32, tag="aggr")
                        nc.vector.tensor_tensor(out=aggr[:], in0=num[:],
                                                in1=recip[:], op=ALU.mult)
                        # transpose aggr -> [64, P]
                        ps_t = psum.tile([64, P], F32, tag="ps")
                        nc.tensor.transpose(ps_t[:], aggr[:], ident[:])
                        aggrT = sbuf.tile([64, P], F32, tag="aggrTs")
                        nc.scalar.copy(aggrT[:], ps_t[:])
                        aggrT_ap = aggrT[:]
                    else:
                        aggrT_ap = zero64[:]
                    outT = mlp_block_tile(w, aggrT_ap, xT_t[:])
                    epilogue(t, outT, xT_t)

            def write_nodemajor(t, outT, space, slice_buf, ext_out=None):
                """Transpose outT back to node-major, write slice rows."""
                ps_b = psum.tile([P, 64], F32, tag="ps")
                nc.tensor.transpose(ps_b[:], outT[:], ident[0:64, 0:64])
                nm = sbuf.tile([P, 64], F32, tag="nms")
                nc.scalar.copy(nm[:], ps_b[:])
                lo = t * P
                hi = min(space.nc, lo + P)
                if hi > lo:
                    nc.sync.dma_start(slice_buf[lo:hi, :], nm[0:hi - lo, :])
                    if ext_out is not None:
                        nc.sync.dma_start(ext_out[lo:hi, :], nm[0:hi - lo, :])

            # ---------------- block 1: plane (hit<-hit), src table = xhit_full
            def epi_b1(t, outT, xT_t):
                nc.sync.dma_start(h1_xt[:, t * P:(t + 1) * P], outT[:])
                write_nodemajor(t, outT, hit, h1_slice)

            gather_block("b1", "plane", xhit_full, hit_xt, hit, epi_b1)
            allgather(h1_slice, h1_full)

            # ---------------- block 2: p2n (sp <- h1), dst x = x_sp
            def epi_b2(t, outT, xT_t):
                nc.sync.dma_start(sp2_xt[:, t * P:(t + 1) * P], outT[:])
                write_nodemajor(t, outT, sp, sp2_slice)

            gather_block("b2", "p2n", h1_full, sp_xt, sp, epi_b2)
            allgather(sp2_slice, sp2_full)

            # ---------------- block 3: n2i (evt <- sp2), dst x = x_evt
            def epi_b3(t, outT, xT_t):
                write_nodemajor(t, outT, evt, evt3_slice, ext_out=evt_out)

            gather_block("b3", "n2i", sp2_full, evt_xt, evt, epi_b3)
            allgather(evt3_slice, evt3_full)

            # ---------------- block 4: i2n (sp <- evt3), dst x = sp2
            def epi_b4(t, outT, xT_t):
                write_nodemajor(t, outT, sp, sp4_slice, ext_out=sp_out)

            gather_block("b4", "i2n", evt3_full, sp2_xt, sp, epi_b4)
            allgather(sp4_slice, sp4_full)

            # ---------------- block 5: n2p (hit <- sp4), dst x = h1; + final MLPs
            def epi_b5(t, outT, xT_t):
                # outT = h5 tile [64, P]; xT_t = h1 tile
                nc.sync.dma_start(h_out[:, t * P:(t + 1) * P], outT[:])
                # ---- beta MLP: cat [of(1) | h(64)] -> 192 -> 192 -> 1 sigmoid
                for name, cat_extra, extra_ap, out_ext, final in (
                    ("beta", 1, oft, of_out, "sigmoid"),
                    ("coord", 16, oxt, ox_out, "none"),
                ):
                    w = wsb[name]
                    fe = cat_extra
                    ex = sbuf.tile([fe, P], F32, tag=f"ex{name}")
                    nc.sync.dma_start(ex[:], extra_ap[:, t * P:(t + 1) * P])
                    # layer 1: [fe+64] -> 192, M-split 128+64
                    h1a = sbuf.tile([P, P], F32, tag=f"{name}h1a")
                    h1b = sbuf.tile([64, P], F32, tag=f"{name}h1b")
                    for (mlo, mhi, ht, b1t) in ((0, 128, h1a, w["b1a"]),
                                                (128, 192, h1b, w["b1b"])):
                        pp = psum.tile([mhi - mlo, P], F32, tag="ps")
                        nc.tensor.matmul(pp[:], lhsT=w["W1x"][:, mlo:mhi],
                                         rhs=ex[:], start=True, stop=False)
                        nc.tensor.matmul(pp[:], lhsT=w["W1h"][:, mlo:mhi],
                                         rhs=outT[:], start=False, stop=True)
                        mish_from_psum(pp, b1t[:], ht, mhi - mlo)
                    # layer 2: 192 -> 192
                    h2a = sbuf.tile([P, P], F32, tag=f"{name}h2a")
                    h2b = sbuf.tile([64, P], F32, tag=f"{name}h2b")
                    for (mlo, mhi, ht, b2t) in ((0, 128, h2a, w["b2a"]),
                                                (128, 192, h2b, w["b2b"])):
                        pp = psum.tile([mhi - mlo, P], F32, tag="ps")
                        nc.tensor.matmul(pp[:], lhsT=w["W2a"][:, mlo:mhi],
                                         rhs=h1a[:], start=True, stop=False)
                        nc.tensor.matmul(pp[:], lhsT=w["W2b"][:, mlo:mhi],
                                         rhs=h1b[:], start=False, stop=True)
                        mish_from_psum(pp, b2t[:], ht, mhi - mlo)
                    # layer 3: 192 -> fo
                    fo = Wm[name]["W3a"].shape[1]
                    pp = psum1.tile([fo, P], F32, tag="ps1")
                    nc.tensor.matmul(pp[:], lhsT=w["W3a"][:], rhs=h2a[:],
                                     start=True, stop=False)
                    nc.tensor.matmul(pp[:], lhsT=w["W3b"][:], rhs=h2b[:],
                                     start=False, stop=True)
                    ot = sbuf.tile([fo, P], F32, tag=f"{name}out")
                    nc.scalar.activation(ot[:], pp[:], AF.Identity,
                                         bias=w["b3"][:])
                    if final == "sigmoid":
                        sigmoid_inplace(ot, fo, P)
                    nc.sync.dma_start(out_ext[:, t * P:(t + 1) * P], ot[:])

            gather_block("b5", "n2p", sp4_full, h1_xt, hit, epi_b5)

    nc.finalize()
    return nc


# ---------------------------------------------------------------- runner

def _in_maps(meta, ncores):
    maps = []
    for c in range(ncores):
        m = dict(
            xhit_full=meta["xhit_full"],
            hit_xt=meta["hit_xts"][c],
            sp_xt=meta["sp_xts"][c],
            evt_xt=meta["evt_xts"][c],
            oft=meta["ofts"][c],
            oxt=meta["oxts"][c],
        )
        for b in ("b1", "b2", "b3", "b4", "b5"):
            m[f"idx_{b}"] = meta["blocks"][b][2][c]
        Wm = meta["W"]
        for name in ("plane", "p2n", "n2i", "i2n", "n2p"):
            w = Wm[name]
            m[f"{name}_wedst"] = w["we_dst"]
            m[f"{name}_wesrc"] = w["we_src_bc"]
            for k in ("W1a", "W1b", "b1", "W2", "b2"):
                m[f"{name}_{k}"] = w[k]
        for name in ("beta", "coord"):
            for k in ("W1x", "W1h", "b1a", "b1b", "W2a", "W2b", "b2a", "b2b", "W3a", "W3b", "b3"):
                m[f"{name}_{k}"] = Wm[name][k]
        maps.append(m)
    return maps


def _postprocess(meta, results, sizes):
    hit, sp, evt = meta["hit"], meta["sp"], meta["evt"]
    HIT, SP, EVT = sizes["HIT"], sizes["SP"], sizes["EVT"]
    h = np.empty((HIT, 64), np.float32)
    of_new = np.empty((HIT, 1), np.float32)
    ox_new = np.empty((HIT, 16), np.float32)
    sp_new = np.empty((SP, 64), np.float32)
    evt_new = np.empty((EVT, 64), np.float32)
    for c in range(NCORES):
        r = results[c]
        ids = hit.perm_ids(c)
        h[ids] = r["h_out"][:, : hit.nc].T
        of_new[ids, 0] = r["of_out"][0, : hit.nc]
        ox_new[ids] = r["ox_out"][:, : hit.nc].T
        sp_new[sp.perm_ids(c)] = r["sp_out"]
        evt_new[evt.perm_ids(c)] = r["evt_out"]
    return h, sp_new, evt_new, of_new, ox_new


def run_model(inputs, sizes, ncores=NCORES, use_sim=False):
    """Build (cached), run, and postprocess."""
    meta = _prep(inputs, sizes)
    key = ("nc", tuple(sorted(sizes.items())), ncores,
           tuple(int(meta["blocks"][b][3]) for b in ("b1", "b2", "b3", "b4", "b5")))
    if key not in _CACHE:
        _CACHE[key] = _build_nc(meta, ncores)
    nc = _CACHE[key]
    in_maps = _in_maps(meta, ncores)

    if use_sim:
        from concourse.bass_interp import MultiCoreSim
        sim = MultiCoreSim(nc, num_cores=ncores, require_finite=False,
                           require_nnan=False)
        sims = list(sim.cores.values())
        for c, cs in enumerate(sims):
            for k, v in in_maps[c].items():
                cs.tensor(k)[:] = v
        sim.simulate(check_with_hw=False)
        outs = ("h_out", "sp_out", "evt_out", "of_out", "ox_out")
        results = [{k: np.asarray(cs.tensor(k)) for k in outs} for cs in sims]
    else:
        rkey = ("runner", key)
        if rkey not in _CACHE:
            SpmdRunner = _make_runner_class()
            _CACHE[rkey] = SpmdRunner(nc, ncores)
        r = _CACHE[rkey]
        r.put(in_maps)
        results = r.results(r.run())
    return _postprocess(meta, results, sizes)


def kernel(**inputs):
    sizes = dict(HIT=200000, SP=40000, EVT=512)
    return run_model(inputs, sizes)


# ---------------------------------------------------------------- spmd runner
# (inlined so kernel.py is self-contained)

def _make_runner_class():
    import jax
    from jax.experimental.shard_map import shard_map
    from jax.sharding import Mesh, NamedSharding, PartitionSpec
    import concourse.mybir as mybir
    from concourse.bass2jax import (_bass_exec_p, install_neuronx_cc_hook,
                                    partition_id_tensor)

    class SpmdRunner:
        def __init__(self, nc, n_cores):
            install_neuronx_cc_hook()
            self.nc = nc
            self.n_cores = n_cores
            partition_name = (nc.partition_id_tensor.name
                              if nc.partition_id_tensor else None)
            in_names, out_names, out_avals, zero_outs = [], [], [], []
            for alloc in nc.m.functions[0].allocations:
                if not isinstance(alloc, mybir.MemoryLocationSet):
                    continue
                name = alloc.memorylocations[0].name
                if alloc.kind == "ExternalInput":
                    if name != partition_name:
                        in_names.append(name)
                elif alloc.kind == "ExternalOutput":
                    out_names.append(name)
                    shape = tuple(alloc.tensor_shape)
                    dtype = mybir.dt.np(alloc.dtype)
                    out_avals.append(jax.core.ShapedArray(shape, dtype))
                    zero_outs.append(np.zeros(shape, dtype))
            self.in_names, self.out_names = in_names, out_names
            self.out_avals, self.zero_outs = out_avals, zero_outs
            n_params, n_outs = len(in_names), len(out_avals)
            self.n_params = n_params
            all_in_names = list(in_names) + list(out_names)
            if partition_name is not None:
                all_in_names.append(partition_name)

            def _body(*args):
                operands = list(args)
                if partition_name is not None:
                    operands.append(partition_id_tensor())
                outs = _bass_exec_p.bind(
                    *operands,
                    out_avals=tuple(out_avals),
                    in_names=tuple(all_in_names),
                    out_names=tuple(out_names),
                    lowering_input_output_aliases=(),
                    sim_require_finite=True,
                    sim_require_nnan=True,
                    nc=nc,
                )
                return tuple(outs)

            devices = jax.devices()[:n_cores]
            self.mesh = Mesh(np.asarray(devices), ("core",))
            in_specs = (PartitionSpec("core"),) * (n_params + n_outs)
            out_specs = (PartitionSpec("core"),) * len(out_names)
            self.sharded = jax.jit(
                shard_map(_body, mesh=self.mesh, in_specs=in_specs,
                          out_specs=out_specs, check_rep=False),
                keep_unused=True,
            )
            self.sharding = NamedSharding(self.mesh, PartitionSpec("core"))
            self._dev_in = None
            self._jax = jax

        def put(self, in_maps):
            jax = self._jax
            per_core = [[np.asarray(m[name]) for name in self.in_names]
                        for m in in_maps]
            concat_in = [
                np.concatenate([per_core[c][i] for c in range(self.n_cores)], 0)
                for i in range(self.n_params)
            ]
            concat_zeros = [
                np.zeros((self.n_cores * z.shape[0], *z.shape[1:]), z.dtype)
                for z in self.zero_outs
            ]
            self._dev_in = [jax.device_put(a, self.sharding)
                            for a in concat_in + concat_zeros]

        def run(self):
            outs = self.sharded(*self._dev_in)
            self._jax.block_until_ready(outs)
            return outs

        def results(self, outs):
            res = []
            for c in range(self.n_cores):
                res.append({
                    name: np.asarray(outs[i]).reshape(
                        self.n_cores, *self.out_avals[i].shape)[c]
                    for i, name in enumerate(self.out_names)
                })
            return res

        def time(self, reps=10, warmup=2):
            import time as _time
            for _ in range(warmup):
                self.run()
            ts = []
            for _ in range(reps):
                t0 = _time.perf_counter()
                self.run()
                ts.append(_time.perf_counter() - t0)
            return min(ts), sorted(ts)[len(ts) // 2]

    return SpmdRunner
